# revision 25
# baseline (speedup 1.0000x reference)
# Area-attention (AAttn) kernel for Trainium2, SPMD over 8 NeuronCores.
#
# Reference computation (B=4, C=256, H=W=64, 8 heads, head_dim=32, AREA=4):
#   qk = conv1x1(x, Wqk) + bqk ; v4 = conv1x1(x, Wv) + bv
#   pp = dwconv5x5(v4, Wpe) + bpe          (depthwise, zero pad 2)
#   area split: 16 chunks of 1024 tokens (16 image rows each)
#   attn = softmax(q k^T / sqrt(32)) v     per (chunk, head)
#   out  = conv1x1(attn_out + pp, Wproj) + bproj
#
# Sharding: data-parallel over the 16 (B*area) chunks -> 2 chunks per core,
# i.e. core c handles image b=c//2, rows 32*(c%2)..32*(c%2)+32. No
# collectives; the dwconv halo (2 rows) is shipped with the input shard.
#
# Per-core layouts (prepared in numpy inside kernel()):
#   x      [128, 2, 2448]  bf16  channel-major, ktile axis; 36x68 padded grid
#   vmask  [1, 2448]       bf16  1.0 where the ext-grid cell is a real pixel
#   weights pre-transposed/bf16 so matmul stationaries slice directly.
#
# Inside a core: everything bf16 on the TensorEngine (fp32 psum accum).
# Scores are computed transposed (S^T[k,q]) with 16-way PE tiling
# (4 heads x 4 k-subtiles of 32), exp on ScalarE straight out of PSUM
# (scale=1/sqrt(32) folded in), PV as out^T = [ones|V]^T P^T giving the
# softmax denominator for free, normalization via approx-reciprocal +
# gpsimd partition-broadcast, dwconv as 25 shifted multiply-accumulates
# on VectorE, final 1x1 conv on PE.

import numpy as np
import ml_dtypes

import concourse.bass as bass
import concourse.bacc as bacc
import concourse.mybir as mybir
import concourse.tile as tile
from concourse.bass_utils import run_bass_kernel_spmd
from concourse.masks import make_identity

BF16 = mybir.dt.bfloat16
F32 = mybir.dt.float32
NPBF = ml_dtypes.bfloat16
ALU = mybir.AluOpType
ACT = mybir.ActivationFunctionType

TRACE = False
DEBUG = False
LAST_EXEC_NS = None
LAST_RESULTS = None

EXTR, EXTC = 36, 68
NEXT = EXTR * EXTC        # 2448 ext tokens
NTOK = 2048               # 32x64 center tokens per core
SCALE = float(1.0 / np.sqrt(32.0))

_cached_nc = None



def _build():
    nc = bacc.Bacc()

    x_d = nc.declare_dram_parameter("x", [128, 2, NEXT], BF16, isOutput=False)
    vmask_d = nc.declare_dram_parameter("vmask", [1, NEXT], BF16, isOutput=False)
    wqk_d = nc.declare_dram_parameter("wqk", [128, 2, 512], BF16, isOutput=False)
    wv_d = nc.declare_dram_parameter("wv", [128, 2, 256], BF16, isOutput=False)
    wproj_d = nc.declare_dram_parameter("wproj", [128, 2, 256], BF16, isOutput=False)
    bqk_d = nc.declare_dram_parameter("bqk", [128, 4], F32, isOutput=False)
    bvrow_d = nc.declare_dram_parameter("bvrow", [1, 256], BF16, isOutput=False)
    bproj_d = nc.declare_dram_parameter("bproj", [128, 2], F32, isOutput=False)
    wpe_d = nc.declare_dram_parameter("wpe", [128, 2, 25], F32, isOutput=False)
    bpe_d = nc.declare_dram_parameter("bpe", [128, 2], F32, isOutput=False)
    out_d = nc.declare_dram_parameter("out", [2, 128, NTOK], F32, isOutput=True)
    dbg = {}
    if DEBUG:
        dbg["q"] = nc.declare_dram_parameter("dbg_q", [128, 2, NTOK], BF16, isOutput=True)
        dbg["k"] = nc.declare_dram_parameter("dbg_k", [128, 2, NTOK], BF16, isOutput=True)
        dbg["v4"] = nc.declare_dram_parameter("dbg_v4", [128, 2, NEXT], BF16, isOutput=True)
        dbg["va"] = nc.declare_dram_parameter("dbg_va", [128, 16, 256], BF16, isOutput=True)
        dbg["pt"] = nc.declare_dram_parameter("dbg_pt", [128, 8, 2, 1024], BF16, isOutput=True)
        dbg["outn"] = nc.declare_dram_parameter("dbg_outn", [128, 2, NTOK], BF16, isOutput=True)

    with tile.TileContext(nc) as tc:
        with (
            tc.tile_pool(name="singles", bufs=1) as singles,
            tc.tile_pool(name="ptp", bufs=2) as ptp,
            tc.tile_pool(name="psb", bufs=2, space="PSUM") as psb,
            tc.tile_pool(name="pss", bufs=4, space="PSUM") as pss,
            tc.tile_pool(name="dens", bufs=2) as dens,
            tc.tile_pool(name="rdbs", bufs=2) as rdbs,
            tc.tile_pool(name="ys", bufs=2) as ys,
        ):
            # ---- load everything to SBUF ----
            x_sb = singles.tile([128, 2, NEXT], BF16, tag="x")
            nc.sync.dma_start(out=x_sb, in_=x_d[:, :, :])
            wqk_sb = singles.tile([128, 2, 512], BF16, tag="wqk")
            nc.sync.dma_start(out=wqk_sb, in_=wqk_d[:, :, :])
            wv_sb = singles.tile([128, 2, 256], BF16, tag="wv")
            nc.sync.dma_start(out=wv_sb, in_=wv_d[:, :, :])
            wproj_sb = singles.tile([128, 2, 256], BF16, tag="wproj")
            nc.sync.dma_start(out=wproj_sb, in_=wproj_d[:, :, :])
            bqk_sb = singles.tile([128, 4], F32, tag="bqk")
            nc.sync.dma_start(out=bqk_sb, in_=bqk_d[:, :])
            bvrow_sb = singles.tile([1, 256], BF16, tag="bvrow")
            nc.sync.dma_start(out=bvrow_sb, in_=bvrow_d[:, :])
            bproj_sb = singles.tile([128, 2], F32, tag="bproj")
            nc.sync.dma_start(out=bproj_sb, in_=bproj_d[:, :])
            wpe_sb = singles.tile([128, 2, 25], F32, tag="wpe")
            nc.sync.dma_start(out=wpe_sb, in_=wpe_d[:, :, :])
            bpe_sb = singles.tile([128, 2], F32, tag="bpe")
            nc.sync.dma_start(out=bpe_sb, in_=bpe_d[:, :])
            vmask_sb = singles.tile([1, NEXT], BF16, tag="vmask")
            nc.sync.dma_start(out=vmask_sb, in_=vmask_d[:, :])

            ident = singles.tile([128, 128], BF16, tag="ident")
            make_identity(nc, ident)
            ones32 = singles.tile([128, 32], BF16, tag="ones32")
            nc.vector.memset(ones32[:, :], 1.0)
            ones_r = singles.tile([1, 64], BF16, tag="onesr")
            nc.vector.memset(ones_r[:, :], 1.0)

            # DVE touches: absorb DMA-queue deps so TensorScalar ops (1-wait
            # HW limit) only carry their producer-engine wait.
            scr = singles.tile([128, 64], F32, tag="scr")
            nc.vector.tensor_copy(out=scr[:, 0:4], in_=bqk_sb[:, :])
            nc.vector.tensor_copy(out=scr[:, 4:6], in_=bproj_sb[:, :])
            nc.vector.tensor_copy(out=scr[:, 6:8], in_=bpe_sb[:, :])
            nc.vector.tensor_copy(out=scr[:, 8:58], in_=wpe_sb.rearrange("p a b -> p (a b)"))

            q_sb = singles.tile([128, 2, NTOK], BF16, tag="q")
            k_sb = singles.tile([128, 2, NTOK], BF16, tag="k")
            v4_sb = singles.tile([128, 2, NEXT], BF16, tag="v4")
            va_all = singles.tile([128, 16, 256], BF16, tag="vaall")
            outn_sb = singles.tile([128, 2, NTOK], BF16, tag="outn")

            x_v = x_sb.rearrange("p k (r w) -> p k r w", r=EXTR)
            v4_v = v4_sb.rearrange("p o (r w) -> p o r w", r=EXTR)

            # diagonal stationaries for the PE dwconv: diag(wpe[:, o, t])
            dgs = {}
            for o in range(2):
                for t in range(25):
                    dg = singles.tile([128, 128], BF16, tag=f"dg{o}{t}")
                    nc.vector.tensor_scalar_mul(
                        out=dg, in0=ident, scalar1=wpe_sb[:, o, t:t + 1])
                    dgs[(o, t)] = dg

            # ---- P1: projection emitters (most run as pipeline fillers) ----
            def emit_qk(o, half):
                ps = psb.tile([128, 1024], F32, tag="big", name="qkps")
                for ch in range(2):
                    sl = slice(ch * 512, (ch + 1) * 512)
                    r0 = 2 + 16 * half + 8 * ch
                    for kt in range(2):
                        nc.tensor.matmul(
                            ps[:, sl],
                            wqk_sb[:, kt, o * 128:(o + 1) * 128],
                            x_v[:, kt, r0:r0 + 8, 2:66],
                            start=(kt == 0), stop=(kt == 1),
                        )
                dst = q_sb if o < 2 else k_sb
                nc.vector.tensor_scalar(
                    out=dst[:, o % 2, half * 1024:(half + 1) * 1024], in0=ps[:, :],
                    scalar1=bqk_sb[:, o:o + 1], scalar2=None, op0=ALU.add,
                )

            def emit_v4(o, half):
                n0 = half * 1024
                n1 = min(n0 + 1024, NEXT)
                ps = psb.tile([128, 1024], F32, tag="big", name="v4ps")
                for ch in range((n1 - n0 + 511) // 512):
                    sl = slice(n0 + ch * 512, min(n0 + (ch + 1) * 512, n1))
                    psl = slice(ch * 512, ch * 512 + (sl.stop - sl.start))
                    for kt in range(2):
                        nc.tensor.matmul(
                            ps[:, psl], wv_sb[:, kt, o * 128:(o + 1) * 128],
                            x_sb[:, kt, sl], start=(kt == 0), stop=False,
                        )
                    nc.tensor.matmul(
                        ps[:, psl], bvrow_sb[:, o * 128:(o + 1) * 128],
                        vmask_sb[:, sl], start=False, stop=True,
                    )
                nc.vector.tensor_copy(out=v4_sb[:, o, n0:n1], in_=ps[:, 0:n1 - n0])

            for o in range(4):
                for half in range(2):
                    emit_qk(o, half)
            for o in range(2):
                for half in range(3):
                    emit_v4(o, half)

            # ---- attention: 8 groups of 2 heads, software-pipelined ----
            # fillers for group 0: token-major V blocks (also feeds PV)
            def emit_vtok(blk):
                c, kt = blk // 8, blk % 8
                vps = pss.tile([128, 256], F32, tag="small")
                for rr in range(2):
                    row = 2 + 16 * c + 2 * kt + rr
                    for ki in range(2):
                        nc.tensor.matmul(
                            vps[64 * rr:64 * rr + 64, :],
                            x_v[:, ki, row, 2:66],
                            wv_sb[:, ki, :],
                            start=(ki == 0), stop=False,
                            tile_position=(0, 64 * rr),
                            skip_group_check=True,
                        )
                    nc.tensor.matmul(
                        vps[64 * rr:64 * rr + 64, :],
                        ones_r[:, :],
                        bvrow_sb[:, :],
                        start=False, stop=True,
                        tile_position=(0, 64 * rr),
                        skip_group_check=True,
                    )
                nc.vector.tensor_copy(out=va_all[:, blk, :], in_=vps[:, :])

            # PV chunk for a finished group; pvt holds per-qc psum pairs.
            # All accumulation stays in col strips 0/1 (upper-strip chains
            # corrupt when interleaved with score matmuls); odd groups'
            # results are DMA-shifted to the upper partition half of outn.
            def emit_pv(c, hg2, pt, qc, kt, pvt):
                s0 = 2 * (hg2 % 2)
                if kt == 0:
                    pvt[qc] = (pss.tile([64, 512], F32, tag="small", name="pvA"),
                               pss.tile([64, 512], F32, tag="small", name="pvB"))
                pvA, pvB = pvt[qc]
                o = hg2 // 2
                for hl in range(2):
                    h = 4 * o + s0 + hl
                    nc.tensor.matmul(
                        pvA[32 * hl:32 * hl + 32, :],
                        va_all[:, c * 8 + kt, 32 * h:32 * h + 32],
                        pt[:, kt, hl, qc * 512:qc * 512 + 512],
                        start=(kt == 0), stop=(kt == 7),
                        tile_position=(0, 32 * hl),
                        skip_group_check=True,
                    )
                for hl in range(2):
                    nc.tensor.matmul(
                        pvB[32 * hl:32 * hl + 32, :],
                        ones32[:, :],
                        pt[:, kt, hl, qc * 512:qc * 512 + 512],
                        start=(kt == 0), stop=(kt == 7),
                        tile_position=(0, 32 * hl),
                        skip_group_check=True,
                    )
                if kt == 7:
                    tok = slice(c * 1024 + qc * 512, c * 1024 + qc * 512 + 512)
                    den = dens.tile([64, 512], F32, tag="den")
                    nc.vector.tensor_copy(out=den[:, :], in_=pvB[:, :])
                    rdb = rdbs.tile([64, 512], F32, tag="rdb")
                    nc.vector.reciprocal_approx_fast(out=rdb[:, :], in_=den[:, :])
                    if s0 == 0:
                        nc.vector.scalar_tensor_tensor(
                            out=outn_sb[0:64, o, tok],
                            in0=pvA[:, :], scalar=1.0,
                            in1=rdb[:, :], op0=ALU.mult, op1=ALU.mult,
                        )
                    else:
                        on = dens.tile([64, 512], BF16, tag="onb", name="on")
                        nc.vector.scalar_tensor_tensor(
                            out=on[:, :], in0=pvA[:, :], scalar=1.0,
                            in1=rdb[:, :], op0=ALU.mult, op1=ALU.mult,
                        )
                        nc.sync.dma_start(out=outn_sb[64:128, o, tok], in_=on[:, :])

            prev = None
            from collections import deque
            extra = deque()
            for b in range(16):
                extra.append(lambda b=b: emit_vtok(b))

            groups = [(c, hg2) for c in range(2) for hg2 in range(4)]
            for c, hg2 in groups:
                o, s0 = hg2 // 2, 2 * (hg2 % 2)
                pt = ptp.tile([128, 8, 2, 1024], BF16, tag="pt")
                fillers = deque()
                if prev is not None:
                    pc, phg2, ppt = prev
                    pvt = {}
                    for qc in range(2):
                        for kt in range(8):
                            fillers.append(
                                lambda qc=qc, kt=kt, pc=pc, phg2=phg2, ppt=ppt, pvt=pvt:
                                emit_pv(pc, phg2, ppt, qc, kt, pvt))
                for kt in range(8):
                    for qc in range(2):
                        sc = psb.tile([128, 1024], F32, tag="big")
                        ktok = c * 1024 + kt * 128
                        for i in range(2):
                            nc.tensor.matmul(
                                sc[:, i * 512:(i + 1) * 512],
                                k_sb[32 * (s0 + i):32 * (s0 + i) + 32, o,
                                     ktok:ktok + 128],
                                q_sb[32 * (s0 + i):32 * (s0 + i) + 32, o,
                                     c * 1024 + qc * 512:c * 1024 + qc * 512 + 512],
                                start=True, stop=True,
                                tile_position=(32 * (s0 + i), 0),
                            )
                        nc.scalar.activation(
                            out=pt[:, kt, :, qc * 512:qc * 512 + 512],
                            in_=sc.rearrange("p (i q) -> p i q", i=2),
                            func=ACT.Exp, scale=SCALE,
                        )
                        if fillers:
                            fillers.popleft()()
                        elif extra:
                            extra.popleft()()
                prev = (c, hg2, pt)
            while extra:
                extra.popleft()()

            # PV of the last group
            pc, phg2, ppt = prev
            pvt = {}
            for qc in range(2):
                for kt in range(8):
                    emit_pv(pc, phg2, ppt, qc, kt, pvt)

            if DEBUG:
                nc.sync.dma_start(out=dbg["outn"][:, :, :], in_=outn_sb)
                nc.sync.dma_start(out=dbg["pt"][:, :, :, :], in_=ppt)

            # ---- P3: pp = dwconv5 on PE; rhs = outn + pp + bpe (in place) ----
            for o in range(2):
                for ch in range(4):
                    pp = pss.tile([128, 512], F32, tag="small")
                    for t in range(25):
                        dy, dx = t // 5, t % 5
                        nc.tensor.matmul(
                            pp[:, :], dgs[(o, t)],
                            v4_v[:, o, 8 * ch + dy:8 * ch + dy + 8, dx:dx + 64],
                            start=(t == 0), stop=(t == 24),
                        )
                    nc.vector.scalar_tensor_tensor(
                        out=outn_sb[:, o, ch * 512:(ch + 1) * 512],
                        in0=pp[:, :], scalar=bpe_sb[:, o:o + 1],
                        in1=outn_sb[:, o, ch * 512:(ch + 1) * 512],
                        op0=ALU.add, op1=ALU.add,
                    )
            for o in range(2):
                for half in range(2):
                    ps = psb.tile([128, 1024], F32, tag="big")
                    for ch in range(2):
                        for kt in range(2):
                            nc.tensor.matmul(
                                ps[:, ch * 512:(ch + 1) * 512],
                                wproj_sb[:, kt, o * 128:(o + 1) * 128],
                                outn_sb[:, kt, half * 1024 + ch * 512:
                                        half * 1024 + (ch + 1) * 512],
                                start=(kt == 0), stop=(kt == 1),
                            )
                    y_sb = ys.tile([128, 1024], F32, tag="y")
                    nc.vector.tensor_scalar(
                        out=y_sb[:, :], in0=ps[:, :],
                        scalar1=bproj_sb[:, o:o + 1], scalar2=None, op0=ALU.add,
                    )
                    nc.sync.dma_start(
                        out=out_d[o, :, half * 1024:(half + 1) * 1024], in_=y_sb[:, :])

    nc.compile()
    return nc


def _shards(x, Wqk, bqk, Wv, bv, Wpe, bpe, Wproj, bproj):
    B, C, H, W = x.shape
    wqk = np.ascontiguousarray(
        Wqk.T.reshape(2, 128, 512).transpose(1, 0, 2)).astype(NPBF)
    wv = np.ascontiguousarray(
        Wv.T.reshape(2, 128, 256).transpose(1, 0, 2)).astype(NPBF)
    wproj = np.ascontiguousarray(
        Wproj.T.reshape(2, 128, 256).transpose(1, 0, 2)).astype(NPBF)
    bqks = np.ascontiguousarray(bqk.reshape(4, 128).T).astype(np.float32)
    bvrow = bv.reshape(1, 256).astype(NPBF)
    bprojs = np.ascontiguousarray(bproj.reshape(2, 128).T).astype(np.float32)
    wpe = np.ascontiguousarray(
        Wpe.reshape(256, 25).reshape(2, 128, 25).transpose(1, 0, 2)
    ).astype(np.float32)
    bpes = np.ascontiguousarray(bpe.reshape(2, 128).T).astype(np.float32)

    common = dict(wqk=wqk, wv=wv, wproj=wproj, bqk=bqks, bvrow=bvrow,
                  bproj=bprojs, wpe=wpe, bpe=bpes)

    in_maps = []
    for core in range(8):
        b, half = core // 2, core % 2
        xe = np.zeros((256, EXTR, EXTC), np.float32)
        r0 = half * 32
        lo, hi = max(r0 - 2, 0), min(r0 + 34, 64)
        xe[:, (lo - (r0 - 2)):(hi - (r0 - 2)), 2:66] = x[b, :, lo:hi, :]
        xs = np.ascontiguousarray(
            xe.reshape(2, 128, NEXT).transpose(1, 0, 2)).astype(NPBF)
        vm = np.zeros((EXTR, EXTC), np.float32)
        vm[(lo - (r0 - 2)):(hi - (r0 - 2)), 2:66] = 1.0
        vm = vm.reshape(1, NEXT).astype(NPBF)
        in_maps.append(dict(common, x=xs, vmask=vm))
    return in_maps


def kernel(**inputs):
    global _cached_nc, LAST_EXEC_NS, LAST_RESULTS
    if _cached_nc is None:
        _cached_nc = _build()
    inputs = {k: np.asarray(v) for k, v in inputs.items()}
    in_maps = _shards(**inputs)
    res = run_bass_kernel_spmd(_cached_nc, in_maps, list(range(8)), trace=TRACE)
    LAST_EXEC_NS = res.exec_time_ns
    LAST_RESULTS = res
    out = np.zeros((4, 256, 64, 64), np.float32)
    for core in range(8):
        b, half = core // 2, core % 2
        y = np.asarray(res.results[core]["out"]).reshape(256, 32, 64)
        out[b, :, half * 32:(half + 1) * 32, :] = y
    return out
